# revision 53
# baseline (speedup 1.0000x reference)
"""Multi-head attention Trainium2 kernel (8 NeuronCores), v8.

Sharding: 8 cores = 4 batches x 2 head-halves (tensor parallel on heads).
Each core computes, for its (batch, 8 heads): q/k/v projections over the FULL
sequence, scores/softmax/ctx for its 4 head-pairs, and a partial output
projection against its half of Wo's rows.  The host sums the two partial
outputs per batch (row-sharded Wo => exact).

Engine plan (per core):
  - Tensor: scores as K=64 tile_position quadrant pairs (HW-concurrent,
    emitted adjacently so the second pair's ldweights prefetch).  Ctx for
    (j,sc) is DEFERRED: it runs as a dense back-to-back burst through the
    first half of the NEXT (j,sc)'s score loop (ldweights hidden, exp
    latency never stalls PE); key-tiles 12..15 use fp8e4 DoubleRow matmuls
    (one MM per tile pair).  Q/K/V/O projection groups splice into the
    post-normalize half of each loop.  V is projected once for all 8 heads
    with N=512 matmuls.
  - Scalar: exact exp (ACT) for 14/16 key-tiles (fp8e4 out for t=12..13).
  - Vector: Schraudolph fp8 exp for t=14..15, evacuations, normalize.
    Softmax denominators ride along in the ctx matmul as ones-columns
    (exact: they sum the same quantized weights the ctx matmul uses).

Precision: rel err 1.51e-2 vs the 2e-2 gate (fp8 tiles cost ~2x the bf16
baseline's 7.3e-3).  Bias handling: bq/bk asserted zero; bv/bo corrected
exactly on the host (softmax rows sum to 1).
"""

import os

import numpy as np
import ml_dtypes

B, S, E, H, DH = 4, 2048, 1024, 16, 64
NE = E // 128        # contraction e-tiles
NT = S // 128        # key tiles
NP = 4               # head-pairs per core (8 heads)
NSC = S // 512       # query chunks of 512
HH = 8               # heads per core
NCORES = 8

NBF = 12             # key-tiles 0..NBF-1: bf16 ctx path; rest: fp8 DoubleRow
NTP = (NT - NBF) // 2  # fp8 tile-pairs

# exp engine per key-tile: 'a'=scalar ACT (exact), 'v'=DVE schraudolph.
# Within a 2-tile block the engines run concurrently, so alternate them
# until DVE's evac/normalize budget is spent.
EXP_ENG = {t: "a" for t in range(NT)}
for _t in (14, 15):
    EXP_ENG[_t] = "v"

# schraudolph bf16: exp(s/8) ~ bitcast_bf16(int16(round(s*A + B)))
SCHR_A = float(16.0 / np.log(2.0))
SCHR_B = float(127.0 * 128.0 - 5.8 + 0.5)
# schraudolph fp8e4 variant (DVE f32->i8 rounds to nearest)
SCHR_A8 = float(1.0 / np.log(2.0))
SCHR_B8 = 55.54

_cache = {}


def _build():
    import concourse.mybir as mybir
    import concourse.tile as tile
    from concourse import bacc
    from contextlib import ExitStack

    f32 = mybir.dt.float32
    bf16 = mybir.dt.bfloat16
    f8e4 = mybir.dt.float8e4
    i8 = mybir.dt.int8
    i16 = mybir.dt.int16
    EXP = mybir.ActivationFunctionType.Exp
    MULT = mybir.AluOpType.mult
    ADD = mybir.AluOpType.add
    DR = mybir.MatmulPerfMode.DoubleRow

    nc = bacc.Bacc("TRN2", target_bir_lowering=False, debug=False,
                   num_devices=NCORES)

    xT_d = nc.dram_tensor("xT", [E, S], bf16, kind="ExternalInput")
    wq_d = nc.dram_tensor("wq", [E, 512], bf16, kind="ExternalInput")
    wk_d = nc.dram_tensor("wk", [E, 512], bf16, kind="ExternalInput")
    wv_d = nc.dram_tensor("wv", [E, 512], bf16, kind="ExternalInput")
    wo_d = nc.dram_tensor("wo", [512, E], bf16, kind="ExternalInput")
    out_d = nc.dram_tensor("out", [S, E], bf16, kind="ExternalOutput")

    with tile.TileContext(nc) as tc, ExitStack() as top:
        singles = top.enter_context(tc.tile_pool(name="singles", bufs=1))
        sb_kt = top.enter_context(tc.tile_pool(name="sb_kt", bufs=2))
        sb_pt = top.enter_context(tc.tile_pool(name="sb_pt", bufs=15))
        sb_pt8 = top.enter_context(tc.tile_pool(name="sb_pt8", bufs=5))
        sb_nm = top.enter_context(tc.tile_pool(name="sb_nm", bufs=1))
        sb_out = top.enter_context(tc.tile_pool(name="sb_out", bufs=4))
        ps_s = top.enter_context(tc.tile_pool(name="ps_s", bufs=2, space="PSUM"))
        ps_cd = top.enter_context(tc.tile_pool(name="ps_cd", bufs=2, space="PSUM"))
        ps_qk = top.enter_context(tc.tile_pool(name="ps_qk", bufs=2, space="PSUM"))

        qT_sb = singles.tile([128, NP, S], bf16)
        ctx_sb = singles.tile([128, NP, S], bf16)
        v_sb = singles.tile([128, NBF, HH, 128], bf16)
        v8_sb = (singles.tile([128, NTP, 2, HH, 128], f8e4, name="v8_sb")
                 if NTP else None)
        wv_sb = singles.tile([128, NE, 512], bf16)
        wo_sb = singles.tile([128, NP, E], bf16)
        # per-chunk tiles so matmuls wait on single DMAs, not whole arrays
        xT_t = {(e, q): singles.tile([128, 512], bf16, name=f"xT{e}_{q}")
                for e in range(NE) for q in range(4)}
        wq_t = {e: singles.tile([128, 512], bf16, name=f"wq{e}")
                for e in range(NE)}
        wk_t = {e: singles.tile([128, 512], bf16, name=f"wk{e}")
                for e in range(NE)}

        def xsl(e, c0, c1):
            q = c0 // 512
            return xT_t[(e, q)][:, c0 - q * 512:c1 - q * 512]

        xT_r = xT_d.rearrange("(eo ei) t -> ei eo t", ei=128)
        wq_r = wq_d.rearrange("(eo ei) h -> ei eo h", ei=128)
        wk_r = wk_d.rearrange("(eo ei) h -> ei eo h", ei=128)
        wv_r = wv_d.rearrange("(eo ei) h -> ei eo h", ei=128)
        wo_r = wo_d.rearrange("(ho hi) e -> hi ho e", hi=128)

        # prologue DMAs on three queues, ordered by first use; xT in 512-col
        # chunks so the first q/k groups start after ~1MB, not 2MB
        # j=0 column-blocks of wq/wk first: the first q/k groups need only
        # those 0.25MB each
        for jb in (0, 1, 2, 3):
            for e in range(NE):
                nc.gpsimd.dma_start(out=wq_t[e][:, jb * 128:(jb + 1) * 128],
                                    in_=wq_r[:, e, jb * 128:(jb + 1) * 128])
                nc.scalar.dma_start(out=wk_t[e][:, jb * 128:(jb + 1) * 128],
                                    in_=wk_r[:, e, jb * 128:(jb + 1) * 128])
                if jb == 0:
                    nc.sync.dma_start(out=xT_t[(e, 0)][:],
                                      in_=xT_r[:, e, 0:512])
        for q in (1,):
            for e in range(NE):
                nc.sync.dma_start(out=xT_t[(e, q)][:],
                                  in_=xT_r[:, e, q * 512:(q + 1) * 512])
        for q in (2, 3):
            for e in range(NE):
                nc.scalar.dma_start(out=xT_t[(e, q)][:],
                                    in_=xT_r[:, e, q * 512:(q + 1) * 512])
            if q == 2:
                for e in range(NE):
                    nc.gpsimd.dma_start(out=wv_sb[:, e, :], in_=wv_r[:, e, :])
        for j in range(NP):
            nc.sync.dma_start(out=wo_sb[:, j, :], in_=wo_r[:, j, :])

        # ones columns for the softmax denominators (chunked over key
        # tiles so the big memsets don't head-block prologue evacuations)
        def ones_memset(t0, t1):
            nc.vector.memset(v_sb[:, t0:t1, 0:8:2, 64:128], 1.0)
            nc.vector.memset(v_sb[:, t0:t1, 1:8:2, 0:64], 1.0)

        def ones_memset_f8(p0, p1):
            if not NTP:
                return
            nc.vector.memset(v8_sb[:, p0:p1, :, 0:8:2, 64:128], 1.0)
            nc.vector.memset(v8_sb[:, p0:p1, :, 1:8:2, 0:64], 1.0)

        ones_memset(0, 2)

        kt_tiles = {}

        qk_ps = {}

        def q_group(j, sc, part=2):
            def fn():
                if (j, sc) not in qk_ps:
                    qk_ps[(j, sc)] = ps_qk.tile([128, 512], f32, tag="ps_qk",
                                                name="ps_q")
                ps = qk_ps[(j, sc)]
                es = range(4) if part == 0 else range(4, NE) if part == 1 \
                    else range(NE)
                for e in es:
                    nc.tensor.matmul(
                        ps[:], wq_t[e][:, j * 128:(j + 1) * 128],
                        xsl(e, sc * 512, (sc + 1) * 512),
                        start=(e == 0), stop=(e == NE - 1))
                if part != 0:
                    qk_ps.pop((j, sc))
                    nc.vector.tensor_copy(
                        qT_sb[:, j, sc * 512:(sc + 1) * 512], ps[:])
            return fn

        def k_group(j, tch, part=2):
            def fn():
                if tch == 0 and part != 1:
                    kt_tiles[j] = sb_kt.tile([128, S], bf16, tag="kt", name="kt")
                kt = kt_tiles[j]
                if (j, "k", tch) not in qk_ps:
                    qk_ps[(j, "k", tch)] = ps_qk.tile([128, 512], f32,
                                                      tag="ps_qk", name="ps_k")
                ps = qk_ps[(j, "k", tch)]
                es = range(4) if part == 0 else range(4, NE) if part == 1 \
                    else range(NE)
                for e in es:
                    nc.tensor.matmul(
                        ps[:], wk_t[e][:, j * 128:(j + 1) * 128],
                        xsl(e, tch * 512, (tch + 1) * 512),
                        start=(e == 0), stop=(e == NE - 1))
                if part != 0:
                    qk_ps.pop((j, "k", tch))
                    nc.vector.tensor_copy(kt[:, tch * 512:(tch + 1) * 512], ps[:])
            return fn

        def v_group(t):
            # all 8 heads' V for one key-tile in a single N=512 matmul chain
            def fn():
                ps = ps_qk.tile([128, 512], f32, tag="ps_qk")
                for e in range(NE):
                    nc.tensor.matmul(
                        ps[:], xsl(e, t * 128, (t + 1) * 128),
                        wv_sb[:, e, 0:512],
                        start=(e == 0), stop=(e == NE - 1))
                pv = ps.rearrange("p (j two d) -> p j two d", two=2, d=64)
                if t < NBF:
                    nc.vector.tensor_copy(v_sb[:, t, 0:8:2, 0:64], pv[:, :, 0, :])
                    nc.vector.tensor_copy(v_sb[:, t, 1:8:2, 64:128],
                                          pv[:, :, 1, :])
                else:
                    tp, ko = divmod(t - NBF, 2)
                    nc.vector.tensor_copy(v8_sb[:, tp, ko, 0:8:2, 0:64],
                                          pv[:, :, 0, :])
                    nc.vector.tensor_copy(v8_sb[:, tp, ko, 1:8:2, 64:128],
                                          pv[:, :, 1, :])
            return fn

        def out_group(st, half, tail_idx=None):
            # allocates from ps_cd: free in j3's post-normalize iters where
            # all O-groups run (ps_qk is held by the last loop's inline ctx)
            def fn():
                ps = ps_cd.tile([128, 512], f32, tag="ps_cd")
                for j in range(NP):
                    nc.tensor.matmul(
                        ps[:], ctx_sb[:, j, st * 128:(st + 1) * 128],
                        wo_sb[:, j, half * 512:(half + 1) * 512],
                        start=(j == 0), stop=(j == NP - 1))
                ot = sb_out.tile([128, 512], bf16, tag="out")
                if tail_idx is None:
                    nc.vector.tensor_copy(ot[:], ps[:])
                else:
                    # tail: rotate evacuation engines so the chain parallelizes
                    eng = (nc.scalar, nc.vector)[tail_idx % 2]
                    if tail_idx % 2 == 0:
                        eng.copy(ot[:], ps[:])
                    else:
                        eng.tensor_copy(ot[:], ps[:])
                dma_eng = (nc.sync, nc.gpsimd)[(st * 2 + half) % 2]
                dma_eng.dma_start(
                    out=out_d[st * 128:(st + 1) * 128,
                              half * 512:(half + 1) * 512],
                    in_=ot[:])
            return fn

        # ---- static splice schedule: global iter (j*64 + sc*16 + t) -> work
        sched = {}

        def at(g, fn):
            sched.setdefault(g, []).append(fn)

        # bursts occupy iters 0..8 of every loop; q/k/v/o splices fill 9..15
        at(12, q_group(0, 1, 0))
        at(13, q_group(0, 1, 1))
        at(26, q_group(0, 2, 0))
        at(27, q_group(0, 2, 1))
        at(42, q_group(0, 3, 0))
        at(43, q_group(0, 3, 1))
        at(4, k_group(0, 2, 0))
        at(5, k_group(0, 2, 1))
        at(6, k_group(0, 3, 0))
        at(7, k_group(0, 3, 1))
        for t in range(NT):
            at(t, v_group(t))
        for j in range(1, NP):
            base = (j - 1) * 64
            for tch in range(4):
                host = base + (tch // 2) * 16   # (j-1) sc0 and sc1
                at(host + 9 + 2 * (tch % 2 * 2), k_group(j, tch, 0))
                at(host + 10 + 2 * (tch % 2 * 2), k_group(j, tch, 1))
            for sc in range(NSC):
                host = base + (2 + sc // 2) * 16  # (j-1) sc2 and sc3
                at(host + 9 + 2 * (sc % 2 * 2), q_group(j, sc, 0))
                at(host + 10 + 2 * (sc % 2 * 2), q_group(j, sc, 1))
        # O-proj for sc: j=3's dense normalize(sc) lands at iter
        # 192+(sc+1)*16+8; splice the 8 groups into the following iters
        for sc in range(NSC - 1):
            for i, st in enumerate(range(4 * sc, 4 * sc + 4)):
                at(192 + (sc + 1) * 16 + 9 + i, out_group(st, 0))
                at(192 + (sc + 1) * 16 + min(12 + i, 15), out_group(st, 1))

        def pump(g):
            for fn in sched.pop(g, []):
                fn()

        def ctx_mm(j, t, pt, ps_ca, ps_cb, start, stop):
            nc.tensor.matmul(ps_ca[:], v_sb[:, t, 2 * j, :], pt[:, 0:512],
                             start=start, stop=stop)
            nc.tensor.matmul(ps_cb[:], v_sb[:, t, 2 * j + 1, :],
                             pt[:, 512:1024], start=start, stop=stop)

        def ctx_dr(j, tp, pt8, ps_ca, ps_cb, stop):
            # fp8 DoubleRow: one MM contracts both key-tiles of the pair
            nc.tensor.matmul(ps_ca[:], v8_sb[:, tp, :, 2 * j, :],
                             pt8[:, 0, :, :], start=False, stop=stop,
                             perf_mode=DR)
            nc.tensor.matmul(ps_cb[:], v8_sb[:, tp, :, 2 * j + 1, :],
                             pt8[:, 1, :, :], start=False, stop=stop,
                             perf_mode=DR)

        def normalize(j, sc, ps_ca, ps_cb, c0=0, c1=512):
            w = c1 - c0
            tA = sb_nm.tile([128, 512], f32, tag="tA")
            tB = sb_nm.tile([128, 512], f32, tag="tB")
            nc.vector.tensor_copy(tA[:, c0:c1], ps_ca[:, c0:c1])
            nc.vector.tensor_copy(tB[:, c0:c1], ps_cb[:, c0:c1])
            # head A: denom replicated at rows 64:128; one row -> partition 0,
            # reciprocal, broadcast back to rows 0:64
            rA = sb_nm.tile([1, 512], f32, tag="rA")
            rbA = sb_nm.tile([64, 512], f32, tag="rbA")
            nc.sync.dma_start(out=rA[0:1, c0:c1], in_=tA[64:65, c0:c1])
            nc.vector.reciprocal_approx_fast(rA[0:1, c0:c1], rA[0:1, c0:c1])
            nc.gpsimd.partition_broadcast(rbA[:, c0:c1], rA[0:1, c0:c1])
            nc.vector.tensor_mul(
                ctx_sb[0:64, j, sc * 512 + c0:sc * 512 + c1], tA[0:64, c0:c1],
                rbA[:, c0:c1])
            # head B: denom at row 0 already
            rB = sb_nm.tile([1, 512], f32, tag="rB")
            rbB = sb_nm.tile([128, 512], f32, tag="rbB")
            nc.vector.reciprocal_approx_fast(rB[0:1, c0:c1], tB[0:1, c0:c1])
            nc.gpsimd.partition_broadcast(rbB[:, c0:c1], rB[0:1, c0:c1])
            nc.vector.tensor_mul(
                ctx_sb[64:128, j, sc * 512 + c0:sc * 512 + c1],
                tB[64:128, c0:c1], rbB[64:128, c0:c1])

        # ---- prologue compute: only what the first score block needs; all
        # v-groups and the xT-half1 k-groups splice into the first loop
        q_group(0, 0)()
        ones_memset(2, 6)
        k_group(0, 0)()
        ones_memset(6, 11)
        k_group(0, 1)()
        ones_memset(11, NBF)
        ones_memset_f8(0, NTP)

        # ---- main attention loop with deferred ctx
        # prev = (j, sc, pts, pt8s) whose ctx/normalize runs in the current
        # loop as back-to-back filler (ldweights hide behind in-flight MMs)
        prev = None
        for j in range(NP):
            for sc in range(NSC):
                last = (j == NP - 1 and sc == NSC - 1)
                if last:
                    # inline ctx accumulators live in ps_qk (free by now)
                    ca_l = ps_qk.tile([128, 512], f32, tag="ps_qk", name="ca_l")
                    cb_l = ps_qk.tile([128, 512], f32, tag="ps_qk", name="cb_l")
                if prev is not None:
                    pj, psc, ppts, ppt8s = prev
                    pca = ps_cd.tile([128, 512], f32, tag="ps_cd")
                    pcb = ps_cd.tile([128, 512], f32, tag="ps_cd")
                pts = {}
                pt8s = {}
                for th in range(NT // 2):
                    kt = kt_tiles[j]
                    # prev's ctx finished in block 3; normalize before any
                    # pumped O-group can read its ctx_sb slice
                    if prev is not None and th == 4:
                        normalize(pj, psc, pca, pcb)
                    pss = []
                    for t in (2 * th, 2 * th + 1):
                        ps_sc_t = ps_s.tile([128, 1024], f32, tag="ps_s",
                                            name="ps_sc_t")
                        nc.tensor.matmul(
                            ps_sc_t[:, 0:512], kt[0:64, t * 128:(t + 1) * 128],
                            qT_sb[0:64, j, sc * 512:(sc + 1) * 512],
                            start=True, stop=True, tile_position=(0, 0))
                        nc.tensor.matmul(
                            ps_sc_t[:, 512:1024],
                            kt[64:128, t * 128:(t + 1) * 128],
                            qT_sb[64:128, j, sc * 512:(sc + 1) * 512],
                            start=True, stop=True, tile_position=(64, 0))
                        pss.append(ps_sc_t)
                    for t, ps_sc_t in zip((2 * th, 2 * th + 1), pss):
                        eng = EXP_ENG[t]
                        if j == NP - 1 and t in (1, 3, 5, 9):
                            # j3 loops have no q/k splices and run ACT-bound;
                            # shift four exps to DVE (it idles there)
                            eng = "v"
                        if last and t >= NBF:
                            # parallelize the tail exps across both engines
                            eng = "a" if (t - NBF) % 2 == 0 else "v"
                        if t < NBF:
                            pt = sb_pt.tile([128, 1024], bf16, tag="pt",
                                            name="pt")
                            if eng == "a":
                                nc.scalar.activation(pt[:], ps_sc_t[:], EXP,
                                                     scale=0.125)
                            else:
                                nc.vector.tensor_scalar(pt.bitcast(i16)[:],
                                                        ps_sc_t[:], SCHR_A,
                                                        SCHR_B, MULT, ADD)
                            pts[t] = pt
                        else:
                            tp, ko = divmod(t - NBF, 2)
                            if ko == 0:
                                pt8s[tp] = sb_pt8.tile([128, 2, 2, 512], f8e4,
                                                       tag="pt8", name="pt8")
                            dst = pt8s[tp][:, :, ko, :]
                            if eng == "a":
                                nc.scalar.activation(dst, ps_sc_t[:], EXP,
                                                     scale=0.125)
                            else:
                                nc.vector.tensor_scalar(dst.bitcast(i8),
                                                        ps_sc_t[:], SCHR_A8,
                                                        SCHR_B8, MULT, ADD)
                        pump(j * 64 + sc * 16 + t)
                    # deferred ctx burst of prev (j,sc): four tiles per
                    # block, back-to-back so ldweights stay hidden
                    if prev is not None:
                        for tt in range(4 * th, 4 * th + 4):
                            if tt < NBF:
                                ctx_mm(pj, tt, ppts.pop(tt), pca, pcb,
                                       start=(tt == 0), stop=False)
                            elif tt < NT and tt % 2 == 0:
                                tp = (tt - NBF) // 2
                                ctx_dr(pj, tp, ppt8s.pop(tp), pca, pcb,
                                       stop=(tp == NTP - 1))
                    # inline pipelined ctx for the very last (j,sc)
                    if last and th >= 1:
                        for tl in (2 * th - 2, 2 * th - 1):
                            if tl < NBF:
                                ctx_mm(j, tl, pts.pop(tl), ca_l, cb_l,
                                       start=(tl == 0), stop=False)
                    if last and th == NT // 2 - 1:
                        ctx_dr(j, 0, pt8s.pop(0), ca_l, cb_l, stop=False)
                if last:
                    # inline path covered bf16 tiles and fp8 pair 0
                    ctx_dr(j, NTP - 1, pt8s.pop(NTP - 1), ca_l, cb_l,
                           stop=True)
                    prev = None
                else:
                    prev = (j, sc, pts, pt8s)

        # ---- tail: remaining scheduled work, the last normalize (split by
        # query half so the first output projections start earlier)
        for g in sorted(sched.keys()):
            for fn in sched.pop(g, []):
                fn()
        j, sc = NP - 1, NSC - 1
        normalize(j, sc, ca_l, cb_l, 0, 256)
        out_group(12, 0, tail_idx=0)()
        out_group(12, 1, tail_idx=1)()
        out_group(13, 0, tail_idx=2)()
        normalize(j, sc, ca_l, cb_l, 256, 512)
        out_group(13, 1, tail_idx=3)()
        for i, st in enumerate(range(14, 16)):
            out_group(st, 0, tail_idx=2 * i + 4)()
            out_group(st, 1, tail_idx=2 * i + 5)()

    nc.compile()
    return nc


def _prep(xs, Wq, Wk, Wv, Wo):
    bf = ml_dtypes.bfloat16
    xT_b = [np.ascontiguousarray(xs[b].T).astype(bf) for b in range(B)]
    halves = []
    for g in range(2):
        hsl = slice(g * 8, (g + 1) * 8)
        halves.append({
            "wq": np.ascontiguousarray(
                Wq[hsl].transpose(1, 0, 2).reshape(E, 512)).astype(bf),
            "wk": np.ascontiguousarray(
                Wk[hsl].transpose(1, 0, 2).reshape(E, 512)).astype(bf),
            "wv": np.ascontiguousarray(
                Wv[hsl].transpose(1, 0, 2).reshape(E, 512)).astype(bf),
            "wo": np.ascontiguousarray(Wo[g * 512:(g + 1) * 512]).astype(bf),
        })
    in_maps = []
    for c in range(NCORES):
        b, g = divmod(c, 2)
        m = {"xT": xT_b[b]}
        m.update(halves[g])
        in_maps.append(m)
    return in_maps


def kernel(xs, Wq, bq, Wk, bk, Wv, bv, Wo, bo):
    from concourse.bass_utils import run_bass_kernel_spmd

    if "nc" not in _cache:
        _cache["nc"] = _build()
    nc = _cache["nc"]

    xs = np.asarray(xs, dtype=np.float32)
    Wq = np.asarray(Wq, dtype=np.float32)
    Wk = np.asarray(Wk, dtype=np.float32)
    Wv = np.asarray(Wv, dtype=np.float32)
    Wo = np.asarray(Wo, dtype=np.float32)
    bq = np.asarray(bq, dtype=np.float32)
    bk = np.asarray(bk, dtype=np.float32)
    bv = np.asarray(bv, dtype=np.float32)
    bo = np.asarray(bo, dtype=np.float32)
    assert not (np.any(bq) or np.any(bk)), "nonzero bq/bk not supported"

    in_maps = _prep(xs, Wq, Wk, Wv, Wo)

    trace = bool(int(os.environ.get("BASS_KERNEL_TRACE", "0")))
    if trace:
        try:
            import antenv.axon_hooks  # noqa: F401  (registered by the harness)
        except ImportError:
            trace = False
    kw = dict(trace=True, trace_cores=[0]) if trace else {}
    res = run_bass_kernel_spmd(nc, in_maps, core_ids=list(range(NCORES)), **kw)
    if trace and res.exec_time_ns is not None:
        print(f"HW exec time: {res.exec_time_ns} ns")
        if res.instructions_and_trace is not None:
            print("trace:", res.instructions_and_trace[1])

    out = np.empty((B, S, E), dtype=np.float32)
    for b in range(B):
        out[b] = res.results[2 * b]["out"].astype(np.float32)
        out[b] += res.results[2 * b + 1]["out"].astype(np.float32)

    # exact host-side correction for v/output biases (zero in this problem)
    if np.any(bv) or np.any(bo):
        out += bv.reshape(E) @ Wo + bo
    return out


# revision 54
# speedup vs baseline: 1.0303x; 1.0303x over previous
"""Multi-head attention Trainium2 kernel (8 NeuronCores), v8.

Sharding: 8 cores = 4 batches x 2 head-halves (tensor parallel on heads).
Each core computes, for its (batch, 8 heads): q/k/v projections over the FULL
sequence, scores/softmax/ctx for its 4 head-pairs, and a partial output
projection against its half of Wo's rows.  The host sums the two partial
outputs per batch (row-sharded Wo => exact).

Engine plan (per core):
  - Tensor: scores as K=64 tile_position quadrant pairs (HW-concurrent,
    emitted adjacently so the second pair's ldweights prefetch).  Ctx for
    (j,sc) is DEFERRED: it runs as a dense back-to-back burst through the
    first half of the NEXT (j,sc)'s score loop (ldweights hidden, exp
    latency never stalls PE); key-tiles 12..15 use fp8e4 DoubleRow matmuls
    (one MM per tile pair).  Q/K/V/O projection groups splice into the
    post-normalize half of each loop.  V is projected once for all 8 heads
    with N=512 matmuls.
  - Scalar: exact exp (ACT) for 14/16 key-tiles (fp8e4 out for t=12..13).
  - Vector: Schraudolph fp8 exp for t=14..15, evacuations, normalize.
    Softmax denominators ride along in the ctx matmul as ones-columns
    (exact: they sum the same quantized weights the ctx matmul uses).

Precision: rel err 1.51e-2 vs the 2e-2 gate (fp8 tiles cost ~2x the bf16
baseline's 7.3e-3).  Bias handling: bq/bk asserted zero; bv/bo corrected
exactly on the host (softmax rows sum to 1).
"""

import os

import numpy as np
import ml_dtypes

B, S, E, H, DH = 4, 2048, 1024, 16, 64
NE = E // 128        # contraction e-tiles
NT = S // 128        # key tiles
NP = 4               # head-pairs per core (8 heads)
NSC = S // 512       # query chunks of 512
HH = 8               # heads per core
NCORES = 8

NBF = 12             # key-tiles 0..NBF-1: bf16 ctx path; rest: fp8 DoubleRow
NTP = (NT - NBF) // 2  # fp8 tile-pairs

# exp engine per key-tile: 'a'=scalar ACT (exact), 'v'=DVE schraudolph.
# Within a 2-tile block the engines run concurrently, so alternate them
# until DVE's evac/normalize budget is spent.
EXP_ENG = {t: "a" for t in range(NT)}
for _t in (14, 15):
    EXP_ENG[_t] = "v"

# schraudolph bf16: exp(s/8) ~ bitcast_bf16(int16(round(s*A + B)))
SCHR_A = float(16.0 / np.log(2.0))
SCHR_B = float(127.0 * 128.0 - 5.8 + 0.5)
# schraudolph fp8e4 variant (DVE f32->i8 rounds to nearest)
SCHR_A8 = float(1.0 / np.log(2.0))
SCHR_B8 = 55.54

_cache = {}


def _build():
    import concourse.mybir as mybir
    import concourse.tile as tile
    from concourse import bacc
    from contextlib import ExitStack

    f32 = mybir.dt.float32
    bf16 = mybir.dt.bfloat16
    f8e4 = mybir.dt.float8e4
    i8 = mybir.dt.int8
    i16 = mybir.dt.int16
    EXP = mybir.ActivationFunctionType.Exp
    MULT = mybir.AluOpType.mult
    ADD = mybir.AluOpType.add
    DR = mybir.MatmulPerfMode.DoubleRow

    nc = bacc.Bacc("TRN2", target_bir_lowering=False, debug=False,
                   num_devices=NCORES)

    xT_d = nc.dram_tensor("xT", [E, S], bf16, kind="ExternalInput")
    wq_d = nc.dram_tensor("wq", [E, 512], bf16, kind="ExternalInput")
    wk_d = nc.dram_tensor("wk", [E, 512], bf16, kind="ExternalInput")
    wv_d = nc.dram_tensor("wv", [E, 512], bf16, kind="ExternalInput")
    wo_d = nc.dram_tensor("wo", [512, E], bf16, kind="ExternalInput")
    out_d = nc.dram_tensor("out", [S, E], bf16, kind="ExternalOutput")

    with tile.TileContext(nc) as tc, ExitStack() as top:
        singles = top.enter_context(tc.tile_pool(name="singles", bufs=1))
        sb_kt = top.enter_context(tc.tile_pool(name="sb_kt", bufs=2))
        sb_pt = top.enter_context(tc.tile_pool(name="sb_pt", bufs=15))
        sb_pt8 = top.enter_context(tc.tile_pool(name="sb_pt8", bufs=5))
        sb_nm = top.enter_context(tc.tile_pool(name="sb_nm", bufs=1))
        sb_out = top.enter_context(tc.tile_pool(name="sb_out", bufs=4))
        ps_s = top.enter_context(tc.tile_pool(name="ps_s", bufs=2, space="PSUM"))
        ps_cd = top.enter_context(tc.tile_pool(name="ps_cd", bufs=2, space="PSUM"))
        ps_qk = top.enter_context(tc.tile_pool(name="ps_qk", bufs=2, space="PSUM"))

        qT_sb = singles.tile([128, NP, S], bf16)
        ctx_sb = singles.tile([128, NP, S], bf16)
        v_sb = singles.tile([128, NBF, HH, 128], bf16)
        v8_sb = (singles.tile([128, NTP, 2, HH, 128], f8e4, name="v8_sb")
                 if NTP else None)
        wv_sb = singles.tile([128, NE, 512], bf16)
        wo_sb = singles.tile([128, NP, E], bf16)
        # per-chunk tiles so matmuls wait on single DMAs, not whole arrays
        xT_t = {(e, q): singles.tile([128, 512], bf16, name=f"xT{e}_{q}")
                for e in range(NE) for q in range(4)}
        wq_t = {e: singles.tile([128, 512], bf16, name=f"wq{e}")
                for e in range(NE)}
        wk_t = {e: singles.tile([128, 512], bf16, name=f"wk{e}")
                for e in range(NE)}

        def xsl(e, c0, c1):
            q = c0 // 512
            return xT_t[(e, q)][:, c0 - q * 512:c1 - q * 512]

        xT_r = xT_d.rearrange("(eo ei) t -> ei eo t", ei=128)
        wq_r = wq_d.rearrange("(eo ei) h -> ei eo h", ei=128)
        wk_r = wk_d.rearrange("(eo ei) h -> ei eo h", ei=128)
        wv_r = wv_d.rearrange("(eo ei) h -> ei eo h", ei=128)
        wo_r = wo_d.rearrange("(ho hi) e -> hi ho e", hi=128)

        # prologue DMAs on three queues, ordered by first use; xT in 512-col
        # chunks so the first q/k groups start after ~1MB, not 2MB
        for e in range(NE):
            nc.gpsimd.dma_start(out=wq_t[e][:], in_=wq_r[:, e, :])
            nc.scalar.dma_start(out=wk_t[e][:], in_=wk_r[:, e, :])
            nc.sync.dma_start(out=xT_t[(e, 0)][:], in_=xT_r[:, e, 0:512])
        for q in (1,):
            for e in range(NE):
                nc.sync.dma_start(out=xT_t[(e, q)][:],
                                  in_=xT_r[:, e, q * 512:(q + 1) * 512])
        for q in (2, 3):
            for e in range(NE):
                nc.scalar.dma_start(out=xT_t[(e, q)][:],
                                    in_=xT_r[:, e, q * 512:(q + 1) * 512])
            if q == 2:
                for e in range(NE):
                    nc.gpsimd.dma_start(out=wv_sb[:, e, :], in_=wv_r[:, e, :])
        for j in range(NP):
            nc.sync.dma_start(out=wo_sb[:, j, :], in_=wo_r[:, j, :])

        # ones columns for the softmax denominators (chunked over key
        # tiles so the big memsets don't head-block prologue evacuations)
        def ones_memset(t0, t1):
            nc.vector.memset(v_sb[:, t0:t1, 0:8:2, 64:128], 1.0)
            nc.vector.memset(v_sb[:, t0:t1, 1:8:2, 0:64], 1.0)

        def ones_memset_f8(p0, p1):
            if not NTP:
                return
            nc.vector.memset(v8_sb[:, p0:p1, :, 0:8:2, 64:128], 1.0)
            nc.vector.memset(v8_sb[:, p0:p1, :, 1:8:2, 0:64], 1.0)

        ones_memset(0, 2)

        kt_tiles = {}

        qk_ps = {}

        def q_group(j, sc, part=2):
            def fn():
                if (j, sc) not in qk_ps:
                    qk_ps[(j, sc)] = ps_qk.tile([128, 512], f32, tag="ps_qk",
                                                name="ps_q")
                ps = qk_ps[(j, sc)]
                es = range(4) if part == 0 else range(4, NE) if part == 1 \
                    else range(NE)
                for e in es:
                    nc.tensor.matmul(
                        ps[:], wq_t[e][:, j * 128:(j + 1) * 128],
                        xsl(e, sc * 512, (sc + 1) * 512),
                        start=(e == 0), stop=(e == NE - 1))
                if part != 0:
                    qk_ps.pop((j, sc))
                    nc.vector.tensor_copy(
                        qT_sb[:, j, sc * 512:(sc + 1) * 512], ps[:])
            return fn

        def k_group(j, tch, part=2):
            def fn():
                if tch == 0 and part != 1:
                    kt_tiles[j] = sb_kt.tile([128, S], bf16, tag="kt", name="kt")
                kt = kt_tiles[j]
                if (j, "k", tch) not in qk_ps:
                    qk_ps[(j, "k", tch)] = ps_qk.tile([128, 512], f32,
                                                      tag="ps_qk", name="ps_k")
                ps = qk_ps[(j, "k", tch)]
                es = range(4) if part == 0 else range(4, NE) if part == 1 \
                    else range(NE)
                for e in es:
                    nc.tensor.matmul(
                        ps[:], wk_t[e][:, j * 128:(j + 1) * 128],
                        xsl(e, tch * 512, (tch + 1) * 512),
                        start=(e == 0), stop=(e == NE - 1))
                if part != 0:
                    qk_ps.pop((j, "k", tch))
                    nc.vector.tensor_copy(kt[:, tch * 512:(tch + 1) * 512], ps[:])
            return fn

        def v_group(t):
            # all 8 heads' V for one key-tile in a single N=512 matmul chain
            def fn():
                ps = ps_qk.tile([128, 512], f32, tag="ps_qk")
                for e in range(NE):
                    nc.tensor.matmul(
                        ps[:], xsl(e, t * 128, (t + 1) * 128),
                        wv_sb[:, e, 0:512],
                        start=(e == 0), stop=(e == NE - 1))
                pv = ps.rearrange("p (j two d) -> p j two d", two=2, d=64)
                if t < NBF:
                    nc.vector.tensor_copy(v_sb[:, t, 0:8:2, 0:64], pv[:, :, 0, :])
                    nc.vector.tensor_copy(v_sb[:, t, 1:8:2, 64:128],
                                          pv[:, :, 1, :])
                else:
                    tp, ko = divmod(t - NBF, 2)
                    nc.vector.tensor_copy(v8_sb[:, tp, ko, 0:8:2, 0:64],
                                          pv[:, :, 0, :])
                    nc.vector.tensor_copy(v8_sb[:, tp, ko, 1:8:2, 64:128],
                                          pv[:, :, 1, :])
            return fn

        def out_group(st, half, tail_idx=None):
            # allocates from ps_cd: free in j3's post-normalize iters where
            # all O-groups run (ps_qk is held by the last loop's inline ctx)
            def fn():
                ps = ps_cd.tile([128, 512], f32, tag="ps_cd")
                for j in range(NP):
                    nc.tensor.matmul(
                        ps[:], ctx_sb[:, j, st * 128:(st + 1) * 128],
                        wo_sb[:, j, half * 512:(half + 1) * 512],
                        start=(j == 0), stop=(j == NP - 1))
                ot = sb_out.tile([128, 512], bf16, tag="out")
                if tail_idx is None:
                    nc.vector.tensor_copy(ot[:], ps[:])
                else:
                    # tail: rotate evacuation engines so the chain parallelizes
                    eng = (nc.scalar, nc.vector)[tail_idx % 2]
                    if tail_idx % 2 == 0:
                        eng.copy(ot[:], ps[:])
                    else:
                        eng.tensor_copy(ot[:], ps[:])
                dma_eng = (nc.sync, nc.gpsimd)[(st * 2 + half) % 2]
                dma_eng.dma_start(
                    out=out_d[st * 128:(st + 1) * 128,
                              half * 512:(half + 1) * 512],
                    in_=ot[:])
            return fn

        # ---- static splice schedule: global iter (j*64 + sc*16 + t) -> work
        sched = {}

        def at(g, fn):
            sched.setdefault(g, []).append(fn)

        # bursts occupy iters 0..8 of every loop; q/k/v/o splices fill 9..15
        at(12, q_group(0, 1, 0))
        at(13, q_group(0, 1, 1))
        at(26, q_group(0, 2, 0))
        at(27, q_group(0, 2, 1))
        at(42, q_group(0, 3, 0))
        at(43, q_group(0, 3, 1))
        at(4, k_group(0, 2, 0))
        at(5, k_group(0, 2, 1))
        at(6, k_group(0, 3, 0))
        at(7, k_group(0, 3, 1))
        for t in range(NT):
            at(t, v_group(t))
        for j in range(1, NP):
            base = (j - 1) * 64
            for tch in range(4):
                host = base + (tch // 2) * 16   # (j-1) sc0 and sc1
                at(host + 9 + 2 * (tch % 2 * 2), k_group(j, tch, 0))
                at(host + 10 + 2 * (tch % 2 * 2), k_group(j, tch, 1))
            for sc in range(NSC):
                host = base + (2 + sc // 2) * 16  # (j-1) sc2 and sc3
                at(host + 9 + 2 * (sc % 2 * 2), q_group(j, sc, 0))
                at(host + 10 + 2 * (sc % 2 * 2), q_group(j, sc, 1))
        # O-proj for sc: j=3's dense normalize(sc) lands at iter
        # 192+(sc+1)*16+8; splice the 8 groups into the following iters
        for sc in range(NSC - 1):
            for i, st in enumerate(range(4 * sc, 4 * sc + 4)):
                at(192 + (sc + 1) * 16 + 9 + i, out_group(st, 0))
                at(192 + (sc + 1) * 16 + min(12 + i, 15), out_group(st, 1))

        def pump(g):
            for fn in sched.pop(g, []):
                fn()

        def ctx_mm(j, t, pt, ps_ca, ps_cb, start, stop):
            nc.tensor.matmul(ps_ca[:], v_sb[:, t, 2 * j, :], pt[:, 0:512],
                             start=start, stop=stop)
            nc.tensor.matmul(ps_cb[:], v_sb[:, t, 2 * j + 1, :],
                             pt[:, 512:1024], start=start, stop=stop)

        def ctx_dr(j, tp, pt8, ps_ca, ps_cb, stop):
            # fp8 DoubleRow: one MM contracts both key-tiles of the pair
            nc.tensor.matmul(ps_ca[:], v8_sb[:, tp, :, 2 * j, :],
                             pt8[:, 0, :, :], start=False, stop=stop,
                             perf_mode=DR)
            nc.tensor.matmul(ps_cb[:], v8_sb[:, tp, :, 2 * j + 1, :],
                             pt8[:, 1, :, :], start=False, stop=stop,
                             perf_mode=DR)

        def normalize(j, sc, ps_ca, ps_cb, c0=0, c1=512):
            w = c1 - c0
            tA = sb_nm.tile([128, 512], f32, tag="tA")
            tB = sb_nm.tile([128, 512], f32, tag="tB")
            nc.vector.tensor_copy(tA[:, c0:c1], ps_ca[:, c0:c1])
            nc.vector.tensor_copy(tB[:, c0:c1], ps_cb[:, c0:c1])
            # head A: denom replicated at rows 64:128; one row -> partition 0,
            # reciprocal, broadcast back to rows 0:64
            rA = sb_nm.tile([1, 512], f32, tag="rA")
            rbA = sb_nm.tile([64, 512], f32, tag="rbA")
            nc.sync.dma_start(out=rA[0:1, c0:c1], in_=tA[64:65, c0:c1])
            nc.vector.reciprocal_approx_fast(rA[0:1, c0:c1], rA[0:1, c0:c1])
            nc.gpsimd.partition_broadcast(rbA[:, c0:c1], rA[0:1, c0:c1])
            nc.vector.tensor_mul(
                ctx_sb[0:64, j, sc * 512 + c0:sc * 512 + c1], tA[0:64, c0:c1],
                rbA[:, c0:c1])
            # head B: denom at row 0 already
            rB = sb_nm.tile([1, 512], f32, tag="rB")
            rbB = sb_nm.tile([128, 512], f32, tag="rbB")
            nc.vector.reciprocal_approx_fast(rB[0:1, c0:c1], tB[0:1, c0:c1])
            nc.gpsimd.partition_broadcast(rbB[:, c0:c1], rB[0:1, c0:c1])
            nc.vector.tensor_mul(
                ctx_sb[64:128, j, sc * 512 + c0:sc * 512 + c1],
                tB[64:128, c0:c1], rbB[64:128, c0:c1])

        # ---- prologue compute: only what the first score block needs; all
        # v-groups and the xT-half1 k-groups splice into the first loop
        q_group(0, 0)()
        ones_memset(2, 6)
        k_group(0, 0)()
        ones_memset(6, 11)
        k_group(0, 1)()
        ones_memset(11, NBF)
        ones_memset_f8(0, NTP)

        # ---- main attention loop with deferred ctx
        # prev = (j, sc, pts, pt8s) whose ctx/normalize runs in the current
        # loop as back-to-back filler (ldweights hide behind in-flight MMs)
        prev = None
        for j in range(NP):
            for sc in range(NSC):
                last = (j == NP - 1 and sc == NSC - 1)
                if last:
                    # inline ctx accumulators live in ps_qk (free by now)
                    ca_l = ps_qk.tile([128, 512], f32, tag="ps_qk", name="ca_l")
                    cb_l = ps_qk.tile([128, 512], f32, tag="ps_qk", name="cb_l")
                if prev is not None:
                    pj, psc, ppts, ppt8s = prev
                    pca = ps_cd.tile([128, 512], f32, tag="ps_cd")
                    pcb = ps_cd.tile([128, 512], f32, tag="ps_cd")
                pts = {}
                pt8s = {}
                for th in range(NT // 2):
                    kt = kt_tiles[j]
                    # prev's ctx finished in block 3; normalize before any
                    # pumped O-group can read its ctx_sb slice
                    if prev is not None and th == 4:
                        normalize(pj, psc, pca, pcb)
                    pss = []
                    for t in (2 * th, 2 * th + 1):
                        ps_sc_t = ps_s.tile([128, 1024], f32, tag="ps_s",
                                            name="ps_sc_t")
                        nc.tensor.matmul(
                            ps_sc_t[:, 0:512], kt[0:64, t * 128:(t + 1) * 128],
                            qT_sb[0:64, j, sc * 512:(sc + 1) * 512],
                            start=True, stop=True, tile_position=(0, 0))
                        nc.tensor.matmul(
                            ps_sc_t[:, 512:1024],
                            kt[64:128, t * 128:(t + 1) * 128],
                            qT_sb[64:128, j, sc * 512:(sc + 1) * 512],
                            start=True, stop=True, tile_position=(64, 0))
                        pss.append(ps_sc_t)
                    for t, ps_sc_t in zip((2 * th, 2 * th + 1), pss):
                        eng = EXP_ENG[t]
                        if j == NP - 1 and t in (1, 3, 5, 9):
                            # j3 loops have no q/k splices and run ACT-bound;
                            # shift four exps to DVE (it idles there)
                            eng = "v"
                        if last and t >= NBF:
                            # parallelize the tail exps across both engines
                            eng = "a" if (t - NBF) % 2 == 0 else "v"
                        if t < NBF:
                            pt = sb_pt.tile([128, 1024], bf16, tag="pt",
                                            name="pt")
                            if eng == "a":
                                nc.scalar.activation(pt[:], ps_sc_t[:], EXP,
                                                     scale=0.125)
                            else:
                                nc.vector.tensor_scalar(pt.bitcast(i16)[:],
                                                        ps_sc_t[:], SCHR_A,
                                                        SCHR_B, MULT, ADD)
                            pts[t] = pt
                        else:
                            tp, ko = divmod(t - NBF, 2)
                            if ko == 0:
                                pt8s[tp] = sb_pt8.tile([128, 2, 2, 512], f8e4,
                                                       tag="pt8", name="pt8")
                            dst = pt8s[tp][:, :, ko, :]
                            if eng == "a":
                                nc.scalar.activation(dst, ps_sc_t[:], EXP,
                                                     scale=0.125)
                            else:
                                nc.vector.tensor_scalar(dst.bitcast(i8),
                                                        ps_sc_t[:], SCHR_A8,
                                                        SCHR_B8, MULT, ADD)
                        pump(j * 64 + sc * 16 + t)
                    # deferred ctx burst of prev (j,sc): four tiles per
                    # block, back-to-back so ldweights stay hidden
                    if prev is not None:
                        for tt in range(4 * th, 4 * th + 4):
                            if tt < NBF:
                                ctx_mm(pj, tt, ppts.pop(tt), pca, pcb,
                                       start=(tt == 0), stop=False)
                            elif tt < NT and tt % 2 == 0:
                                tp = (tt - NBF) // 2
                                ctx_dr(pj, tp, ppt8s.pop(tp), pca, pcb,
                                       stop=(tp == NTP - 1))
                    # inline pipelined ctx for the very last (j,sc)
                    if last and th >= 1:
                        for tl in (2 * th - 2, 2 * th - 1):
                            if tl < NBF:
                                ctx_mm(j, tl, pts.pop(tl), ca_l, cb_l,
                                       start=(tl == 0), stop=False)
                    if last and th == NT // 2 - 1:
                        ctx_dr(j, 0, pt8s.pop(0), ca_l, cb_l, stop=False)
                if last:
                    # inline path covered bf16 tiles and fp8 pair 0
                    ctx_dr(j, NTP - 1, pt8s.pop(NTP - 1), ca_l, cb_l,
                           stop=True)
                    prev = None
                else:
                    prev = (j, sc, pts, pt8s)

        # ---- tail: remaining scheduled work, the last normalize (split by
        # query half so the first output projections start earlier)
        for g in sorted(sched.keys()):
            for fn in sched.pop(g, []):
                fn()
        j, sc = NP - 1, NSC - 1
        normalize(j, sc, ca_l, cb_l, 0, 256)
        out_group(12, 0, tail_idx=0)()
        out_group(12, 1, tail_idx=1)()
        out_group(13, 0, tail_idx=2)()
        normalize(j, sc, ca_l, cb_l, 256, 512)
        out_group(13, 1, tail_idx=3)()
        for i, st in enumerate(range(14, 16)):
            out_group(st, 0, tail_idx=2 * i + 4)()
            out_group(st, 1, tail_idx=2 * i + 5)()

    nc.compile()
    return nc


def _prep(xs, Wq, Wk, Wv, Wo):
    bf = ml_dtypes.bfloat16
    xT_b = [np.ascontiguousarray(xs[b].T).astype(bf) for b in range(B)]
    halves = []
    for g in range(2):
        hsl = slice(g * 8, (g + 1) * 8)
        halves.append({
            "wq": np.ascontiguousarray(
                Wq[hsl].transpose(1, 0, 2).reshape(E, 512)).astype(bf),
            "wk": np.ascontiguousarray(
                Wk[hsl].transpose(1, 0, 2).reshape(E, 512)).astype(bf),
            "wv": np.ascontiguousarray(
                Wv[hsl].transpose(1, 0, 2).reshape(E, 512)).astype(bf),
            "wo": np.ascontiguousarray(Wo[g * 512:(g + 1) * 512]).astype(bf),
        })
    in_maps = []
    for c in range(NCORES):
        b, g = divmod(c, 2)
        m = {"xT": xT_b[b]}
        m.update(halves[g])
        in_maps.append(m)
    return in_maps


def kernel(xs, Wq, bq, Wk, bk, Wv, bv, Wo, bo):
    from concourse.bass_utils import run_bass_kernel_spmd

    if "nc" not in _cache:
        _cache["nc"] = _build()
    nc = _cache["nc"]

    xs = np.asarray(xs, dtype=np.float32)
    Wq = np.asarray(Wq, dtype=np.float32)
    Wk = np.asarray(Wk, dtype=np.float32)
    Wv = np.asarray(Wv, dtype=np.float32)
    Wo = np.asarray(Wo, dtype=np.float32)
    bq = np.asarray(bq, dtype=np.float32)
    bk = np.asarray(bk, dtype=np.float32)
    bv = np.asarray(bv, dtype=np.float32)
    bo = np.asarray(bo, dtype=np.float32)
    assert not (np.any(bq) or np.any(bk)), "nonzero bq/bk not supported"

    in_maps = _prep(xs, Wq, Wk, Wv, Wo)

    trace = bool(int(os.environ.get("BASS_KERNEL_TRACE", "0")))
    if trace:
        try:
            import antenv.axon_hooks  # noqa: F401  (registered by the harness)
        except ImportError:
            trace = False
    kw = dict(trace=True, trace_cores=[0]) if trace else {}
    res = run_bass_kernel_spmd(nc, in_maps, core_ids=list(range(NCORES)), **kw)
    if trace and res.exec_time_ns is not None:
        print(f"HW exec time: {res.exec_time_ns} ns")
        if res.instructions_and_trace is not None:
            print("trace:", res.instructions_and_trace[1])

    out = np.empty((B, S, E), dtype=np.float32)
    for b in range(B):
        out[b] = res.results[2 * b]["out"].astype(np.float32)
        out[b] += res.results[2 * b + 1]["out"].astype(np.float32)

    # exact host-side correction for v/output biases (zero in this problem)
    if np.any(bv) or np.any(bo):
        out += bv.reshape(E) @ Wo + bo
    return out
